# revision 19
# baseline (speedup 1.0000x reference)
"""Multi-head causal self-attention (RoPE) on 8 TRN2 NeuronCores.

Strategy (tensor-parallel over heads, per the sharding hint):
  - 16 heads / 8 cores -> 2 heads per core. Each core processes ALL 4
    batches for its 2 heads:
      qkv slice -> RoPE -> causal softmax(q k^T) v -> partial out-proj
    and writes a full-shape partial y (row-parallel w_proj). The host
    sums the 8 partials and adds b_proj.
  - x is sent pre-transposed (x^T, [C, T] per batch) so the contraction
    dim C lands on SBUF partitions with no on-device transposes.
  - All attention matmuls run in the "S^T" layout (k-tokens on
    partitions, q-tokens on the free dim):
      S^T tile   = matmul(lhsT=k^T[:,ktile], rhs=q^T[:,qchunk])
      P^T        = exp(S^T * 1/sqrt(D))      (ACT, no max-subtraction:
                                              |scores| <~ 6 so exp is safe)
      denom      = matmul(lhsT=ones[128,1], rhs=P^T)   (partition sum)
      out^T      = matmul(lhsT=v[ktile,:], rhs=P^T)    accumulated
      attn_out^T = out^T * partition_broadcast(1/denom)
    attn_out^T ([d, tok]) is directly the lhsT of the out-projection.
  - RoPE: the head dim d sits on partitions; rotate_half needs rows
    d <-> d+-64. We permute the d index on the host (within each head's
    128 columns of w_qkv + the cos/sin tables) so that rotation partners
    sit 16 apart inside the same 32-partition quadrant, which a single
    DVE stream_shuffle implements. Scores are invariant to the (shared)
    q/k permutation.
"""

import os
from contextlib import ExitStack

import numpy as np

import concourse.bacc as bacc
import concourse.bass as bass
import concourse.mybir as mybir
import concourse.tile as tile
from concourse.bass import ds, ts

B = 4
T = 2048
C = 2048
H = 16
D = 128
NCORES = 8
HPC = H // NCORES  # heads per core = 2
KC = C // 128  # 16 contraction tiles
TT = T // 128  # 16 token tiles
ACH = 256  # stage-A token chunk
NACH = T // ACH
QCH = 512  # stage-B q chunk
NQCH = T // QCH
INV_SQRT_D = float(1.0 / np.sqrt(np.float32(D)))

F32 = mybir.dt.float32
# matmul compute dtype: float32r streams 1 row/cycle (vs 4 for float32)
# when the moving free dim is >= 256.
MM_DT = {
    "f32": mybir.dt.float32,
    "f32r": mybir.dt.float32r,
}[os.environ.get("BASSMHA_MMDT", "f32r")]

# d-permutation: quadrant s holds original d = s*16..s*16+15 (rows 0-15)
# and d+64 partners (rows 16-31); swap = stream_shuffle by +-16.
PERM = np.concatenate(
    [np.concatenate([np.arange(s * 16, s * 16 + 16), 64 + np.arange(s * 16, s * 16 + 16)]) for s in range(4)]
).astype(np.int64)
SWAP_MASK = [(i + 16) % 32 for i in range(32)]


def _mm(nc, out, lhsT, rhs, **kw):
    nc.tensor.matmul(out, lhsT, rhs, **kw)


def build_program():
    nc = bacc.Bacc("TRN2", target_bir_lowering=False, debug=False, num_devices=NCORES)

    xt = nc.dram_tensor("xt", [B, KC, 128, T], MM_DT, kind="ExternalInput").ap()
    wqk = nc.dram_tensor("wqk", [KC, 128, 4 * 128], MM_DT, kind="ExternalInput").ap()
    wv = nc.dram_tensor("wv", [KC, 128, 2 * 128], MM_DT, kind="ExternalInput").ap()
    wproj = nc.dram_tensor("wproj", [HPC, 128, C], MM_DT, kind="ExternalInput").ap()
    cos_in = nc.dram_tensor("cos_t", [128, T], F32, kind="ExternalInput").ap()
    sin_in = nc.dram_tensor("sin_t", [128, T], F32, kind="ExternalInput").ap()
    masks = nc.dram_tensor("masks", [4, 128, QCH], F32, kind="ExternalInput").ap()
    ones_in = nc.dram_tensor("ones", [128, 1], MM_DT, kind="ExternalInput").ap()
    y = nc.dram_tensor("y", [B, TT, 128, C], F32, kind="ExternalOutput").ap()

    with TileKernel(nc) as tk:
        tk.build(xt, wqk, wv, wproj, cos_in, sin_in, masks, ones_in, y)
    nc.compile()
    return nc


class TileKernel:
    def __init__(self, nc):
        self.nc = nc
        self.stack = ExitStack()

    def __enter__(self):
        self.tc = self.stack.enter_context(tile.TileContext(self.nc))
        return self

    def __exit__(self, *exc):
        return self.stack.__exit__(*exc)

    def build(self, xt, wqk, wv, wproj, cos_in, sin_in, masks, ones_in, y):
        nc, tc = self.nc, self.tc
        ctx = self.stack
        from concourse import library_config
        nc.gpsimd.load_library(library_config.attn)

        consts = ctx.enter_context(tc.tile_pool(name="consts", bufs=1))
        store = ctx.enter_context(tc.tile_pool(name="store", bufs=1))
        xtp = ctx.enter_context(tc.tile_pool(name="xtp", bufs=6))
        ropep = ctx.enter_context(tc.tile_pool(name="ropep", bufs=3))
        pp = ctx.enter_context(tc.tile_pool(name="pp", bufs=6))
        rp = ctx.enter_context(tc.tile_pool(name="rp", bufs=2))
        evp = ctx.enter_context(tc.tile_pool(name="evp", bufs=3))

        # ---- persistent weights / tables ----
        wqk_sb = consts.tile([128, KC, 512], MM_DT)
        wv_sb = consts.tile([128, KC, 256], MM_DT)
        wproj_sb = consts.tile([128, HPC, C], MM_DT)
        cos_sb = consts.tile([128, T], F32)
        sin_sb = consts.tile([128, T], F32)
        mask_sb = consts.tile([128, 4, QCH], F32)
        ones_col = consts.tile([128, 1], MM_DT)
        for kc in range(KC):
            nc.sync.dma_start(out=wqk_sb[:, kc, :], in_=wqk[kc])
            nc.sync.dma_start(out=wv_sb[:, kc, :], in_=wv[kc])
        for h in range(HPC):
            nc.sync.dma_start(out=wproj_sb[:, h, :], in_=wproj[h])
        nc.sync.dma_start(out=cos_sb, in_=cos_in)
        nc.sync.dma_start(out=sin_sb, in_=sin_in)
        for r in range(4):
            nc.sync.dma_start(out=mask_sb[:, r, :], in_=masks[r])
        nc.sync.dma_start(out=ones_col, in_=ones_in)

        # ---- per-batch stores ----
        q_t = [store.tile([128, T], MM_DT, name=f"q_t{h}") for h in range(HPC)]
        k_t = [store.tile([128, T], MM_DT, name=f"k_t{h}") for h in range(HPC)]
        v_sb = [store.tile([128, TT, 128], MM_DT, name=f"v_sb{h}") for h in range(HPC)]
        ao_t = [store.tile([128, T], MM_DT, name=f"ao_t{h}") for h in range(HPC)]

        for b in range(B):
            self._stage_a(b, xt, wqk_sb, wv_sb, cos_sb, sin_sb, q_t, k_t, v_sb, xtp, ropep)
            for h in range(HPC):
                self._stage_b(h, q_t, k_t, v_sb, ao_t, mask_sb, ones_col, pp, rp)
            self._stage_c(b, ao_t, wproj_sb, y, evp)

    # qkv projection + RoPE for batch b
    def _stage_a(self, b, xt, wqk_sb, wv_sb, cos_sb, sin_sb, q_t, k_t, v_sb, xtp, ropep):
        nc, tc = self.nc, self.tc
        # Two 256-wide accumulators share each PSUM bank; bufs=2 double-buffers
        # chunks so the PE never waits on the RoPE/v evictions.
        with tc.tile_pool(name=f"psA{b}", bufs=2, space="PSUM") as psA:
            for c in range(NACH):
                seg = ds(c * ACH, ACH)
                xt_tiles = []
                for kc in range(KC):
                    xt_kc = xtp.tile([128, ACH], MM_DT, tag="xt", name=f"xt_{b}_{c}_{kc}")
                    nc.sync.dma_start(out=xt_kc, in_=xt[b, kc, :, seg])
                    xt_tiles.append(xt_kc)
                ps_b = [psA.tile([128, 2 * ACH], F32, tag=f"qkb{p}", name=f"psqkb{p}") for p in range(2)]
                ps_vb = psA.tile([128, 2 * 256], F32, tag="vb", name="psvb")
                ps_qk = [ps_b[m // 2][:, ds((m % 2) * ACH, ACH)] for m in range(4)]
                ps_v = [ps_vb[:, ds(t * 256, 256)] for t in range(ACH // 128)]
                # the two accumulators in one bank form a single group:
                # start zeroes the whole 2KB region, so only the first
                # matmul into a bank starts and only the last stops.
                for kc in range(KC):
                    for m in range(4):
                        _mm(nc, ps_qk[m], wqk_sb[:, kc, ds(m * 128, 128)], xt_tiles[kc],
                            start=(kc == 0 and m % 2 == 0), stop=(kc == KC - 1 and m % 2 == 1))
                    for t in range(ACH // 128):
                        _mm(nc, ps_v[t], xt_tiles[kc][:, ds(t * 128, 128)], wv_sb[:, kc, :],
                            start=(kc == 0 and t == 0), stop=(kc == KC - 1 and t == 1))
                # RoPE eviction: m -> (q/k, head)
                for m in range(4):
                    h = m % 2
                    dst = (q_t if m < 2 else k_t)[h]
                    sw = ropep.tile([128, ACH], F32, tag="sw", name="sw")
                    t1 = ropep.tile([128, ACH], F32, tag="t1", name="t1")
                    nc.vector.stream_shuffle(sw, ps_qk[m], mask=SWAP_MASK)
                    nc.vector.tensor_mul(t1, ps_qk[m], cos_sb[:, seg])
                    nc.vector.tensor_mul(sw, sw, sin_sb[:, seg])
                    nc.vector.tensor_add(dst[:, seg], t1, sw)
                for t in range(ACH // 128):
                    for h in range(HPC):
                        nc.scalar.copy(v_sb[h][:, c * (ACH // 128) + t, :], ps_v[t][:, ds(h * 128, 128)])

    # causal attention for head h (current batch): fills ao_t[h]
    def _stage_b(self, h, q_t, k_t, v_sb, ao_t, mask_sb, ones_col, pp, rp):
        nc, tc = self.nc, self.tc
        with (
            tc.tile_pool(name=f"psS{h}", bufs=3, space="PSUM") as psS,
            tc.tile_pool(name=f"psD{h}", bufs=3, space="PSUM") as psD,
            tc.tile_pool(name=f"psO{h}", bufs=2, space="PSUM") as psO,
        ):
            for jc in range(NQCH):
                qseg = ds(jc * QCH, QCH)
                nvalid = (jc + 1) * (QCH // 128)
                ps_d = psD.tile([1, QCH], F32, tag="d", name="ps_d")
                ps_o = psO.tile([128, QCH], F32, tag="o", name="ps_o")
                # software pipeline: den/out consume ptile two iterations
                # behind the S-matmul/exp/mask producers so the PE never
                # waits on ACT/DVE.
                LAG = 3
                ptiles = {}
                for i in range(nvalid + LAG):
                    if i < nvalid:
                        ps_s = psS.tile([128, QCH], F32, tag="s", name="ps_s")
                        _mm(nc, ps_s, k_t[h][:, ds(i * 128, 128)], q_t[h][:, qseg])
                        ptile = pp.tile([128, QCH], MM_DT, tag="pt", name="ptile")
                        nc.scalar.activation(ptile, ps_s, mybir.ActivationFunctionType.Exp, scale=INV_SQRT_D)
                        r = i - (nvalid - QCH // 128)
                        if r >= 0:
                            # Pool engine: keeps the mask off the busy DVE queue
                            nc.gpsimd.tensor_mul(ptile, ptile, mask_sb[:, r, :])
                        ptiles[i] = ptile
                    j = i - LAG
                    if j >= 0:
                        pt = ptiles.pop(j)
                        _mm(nc, ps_d, ones_col, pt, start=(j == 0), stop=(j == nvalid - 1))
                        _mm(nc, ps_o, v_sb[h][:, j, :], pt, start=(j == 0), stop=(j == nvalid - 1))
                r_sb = rp.tile([1, QCH], F32, tag="r", name="r_sb")
                nc.vector.reciprocal_approx_fast(out=r_sb, in_=ps_d)
                rbc = rp.tile([128, QCH], F32, tag="rbc", name="rbc")
                nc.gpsimd.partition_broadcast(rbc, r_sb)
                nc.vector.tensor_mul(ao_t[h][:, qseg], ps_o, rbc)

    # out-projection partial for batch b
    def _stage_c(self, b, ao_t, wproj_sb, y, evp):
        nc, tc = self.nc, self.tc
        with tc.tile_pool(name=f"psY{b}", bufs=4, space="PSUM") as psY:
            for tt in range(TT):
                for nck in range(C // 512):
                    ps_y = psY.tile([128, 512], F32, tag="y", name="ps_y")
                    for h in range(HPC):
                        _mm(nc, ps_y, ao_t[h][:, ds(tt * 128, 128)], wproj_sb[:, h, ds(nck * 512, 512)],
                            start=(h == 0), stop=(h == HPC - 1))
                    yv = evp.tile([128, 512], F32, tag="yv", name="yv")
                    # alternate eviction engine: ACT alone can't keep pace
                    if nck % 2 == 0:
                        nc.scalar.copy(yv, ps_y)
                    else:
                        nc.vector.tensor_copy(yv, ps_y)
                    nc.sync.dma_start(out=y[b, tt, :, ds(nck * 512, 512)], in_=yv)


def prep_inputs(x, w_qkv, w_proj):
    """Host-side sharding: returns the per-core input maps."""
    x = np.asarray(x, dtype=np.float32)
    w_qkv = np.asarray(w_qkv, dtype=np.float32)
    w_proj = np.asarray(w_proj, dtype=np.float32)

    # x^T per batch: [B, C, T] -> tiled [B, KC, 128, T]
    xt = np.ascontiguousarray(x.transpose(0, 2, 1)).reshape(B, KC, 128, T)

    # RoPE tables (mirror the fp32 reference computation)
    inv_freq = (1.0 / (10000.0 ** (np.arange(0, D, 2, dtype=np.float32) / D))).astype(np.float32)
    t = np.arange(T, dtype=np.float32)
    freqs = np.einsum("i,j->ij", t, inv_freq).astype(np.float32)  # [T, 64]
    emb = np.concatenate([freqs, freqs], axis=-1)  # [T, 128]
    cos_full = np.cos(emb).astype(np.float32)  # [T, 128]
    sin_full = np.sin(emb).astype(np.float32)
    sgn = np.where(np.arange(D) < D // 2, np.float32(-1.0), np.float32(1.0))
    cos_t = np.ascontiguousarray(cos_full[:, PERM].T)  # [128, T]
    sin_t = np.ascontiguousarray((sin_full * sgn)[:, PERM].T)

    # causal masks for the 4 diagonal sub-tiles of a 512-wide q chunk
    kp = np.arange(128)[:, None]
    qf = np.arange(QCH)[None, :]
    masks = np.stack([(qf >= (128 * r + kp)).astype(np.float32) for r in range(4)])

    in_maps = []
    for g in range(NCORES):
        heads = [HPC * g + h for h in range(HPC)]
        # wqk: [C, 512] cols = [q_h0, q_h1, k_h0, k_h1], d-permuted
        cols = []
        for base in (0, C):  # q block, k block
            for hh in heads:
                cols.append(w_qkv[:, base + hh * 128 + PERM])
        wqk_g = np.ascontiguousarray(np.concatenate(cols, axis=1)).reshape(KC, 128, 512)
        wv_g = np.ascontiguousarray(
            np.concatenate([w_qkv[:, 2 * C + hh * 128:2 * C + (hh + 1) * 128] for hh in heads], axis=1)
        ).reshape(KC, 128, 256)
        wproj_g = np.ascontiguousarray(
            np.stack([w_proj[hh * 128:(hh + 1) * 128, :] for hh in heads])
        )
        in_maps.append({
            "xt": xt,
            "wqk": wqk_g,
            "wv": wv_g,
            "wproj": wproj_g,
            "cos_t": cos_t,
            "sin_t": sin_t,
            "masks": masks,
            "ones": np.ones((128, 1), dtype=np.float32),
        })
    return in_maps


_NC_CACHE = {}


def get_program():
    key = MM_DT
    if key not in _NC_CACHE:
        _NC_CACHE[key] = build_program()
    return _NC_CACHE[key]


def kernel(x, w_qkv, w_proj, b_proj):
    from concourse import bass_utils

    nc = get_program()
    in_maps = prep_inputs(x, w_qkv, w_proj)
    res = bass_utils.run_bass_kernel_spmd(nc, in_maps, core_ids=list(range(NCORES)))
    acc = None
    for r in res.results:
        part = r["y"].reshape(B, T, C)
        acc = part if acc is None else acc + part
    return (acc + np.asarray(b_proj, dtype=np.float32)).astype(np.float32)
